# revision 14
# baseline (speedup 1.0000x reference)
"""Trainium2 Bass kernel for nn_AttentionHead (B=2, T=2048, C=2048, H=16 heads, D=128).

Sharding: tensor-parallel over heads - 2 heads per NeuronCore (8 cores).
Each core computes qkv for its heads, RoPE, causal softmax attention, and a
partial c_proj contribution; the host sums the 8 partial outputs.

v2 redesign vs baseline (640us -> target ~350us):
  - All matmul operands in bf16 (PSUM still accumulates f32): same PE rate as
    f32r but LDWEIGHTS gets fast-weight-load (4x) so weight loads hide fully;
    DVE rope runs at 2x; DMA/SBUF traffic halves.
  - Softmax denominator matmul uses a [128,128] ones stationary so the sums
    land in PSUM already broadcast across partitions. reciprocal_approx_fast
    on [128,512] + one fused multiply replace the old
    den -> 1-lane reciprocal (3.3us) -> broadcast-matmul -> copy -> mul chain
    that stalled the PE ~2.5us per (head, query-block).
  - Causal diagonal restriction: S/den/pv matmuls on diagonal key-chunks only
    cover query columns >= chunk offset (saves ~20% of attention PE work).
  - 3-deep software pipeline over 256-token half-blocks:
    A(u) qkv+rope | B(u-1) attention | C(u-2) c_proj, so rope/regather DMA and
    softmax tails always hide under matmul work from a neighboring stage.
  - PSUM banks: pool_a 3x[128,512] (qkv accum + pv + c_proj), sg 2x[128,1024]
    (scores), den 1x[128,512] = exactly 8 banks.
"""

import sys

sys.path.insert(0, "/opt/trn_rl_repo")

import ml_dtypes
import numpy as np

import concourse.bass as bass
import concourse.mybir as mybir
import concourse.tile as tile
from concourse import bacc
from concourse.bass_utils import run_bass_kernel_spmd

F32 = mybir.dt.float32
BF16 = mybir.dt.bfloat16
NP_BF16 = np.dtype(ml_dtypes.bfloat16)

B, T, C, H, D = 2, 2048, 2048, 16, 128
NC_CORES = 8
HPC = H // NC_CORES            # heads per core = 2
BT = B * T                     # 4096
NKT = C // 128                 # 16 contraction tiles
TBS = 512                      # query block size (attention granularity)
HBS = 256                      # stage-A half-block token size
NTB_B = T // TBS               # 4 query blocks per batch
INV_SQRT_D = 1.0 / float(np.sqrt(D))

_CACHE = {}


def _build_program():
    nc = bacc.Bacc(None)

    NHB = BT // HBS             # 16 half-blocks globally
    xP = nc.dram_tensor("xP", [NHB, NKT, 128, HBS], BF16, kind="ExternalInput")
    wqk = nc.dram_tensor("wqk", [NKT, 128, 4 * 128], BF16, kind="ExternalInput")
    wv = nc.dram_tensor("wv", [NKT, 128, HPC * D], BF16, kind="ExternalInput")
    wp = nc.dram_tensor("wp", [HPC, C // 512, 128, 512], BF16, kind="ExternalInput")
    cs = nc.dram_tensor("cs", [4, 128, 512], BF16, kind="ExternalInput")
    sn = nc.dram_tensor("sn", [4, 128, 512], BF16, kind="ExternalInput")
    ones_sq = nc.dram_tensor("ones_sq", [128, 128], BF16, kind="ExternalInput")
    out_d = nc.dram_tensor("out", [BT, C], F32, kind="ExternalOutput")

    NU = 2 * NTB_B              # 8 query blocks across both batches

    with tile.TileContext(nc) as tc:
        with (
            tc.tile_pool(name="const", bufs=1) as constp,
            tc.tile_pool(name="xp", bufs=36) as xp,
            tc.tile_pool(name="qk", bufs=2) as qkp,       # QH/KH per batch
            tc.tile_pool(name="vp", bufs=2) as vpool,     # VH per batch
            tc.tile_pool(name="yp", bufs=2) as ypool,     # yT per batch
            tc.tile_pool(name="qke", bufs=4) as qkep,     # evicted qkT/qkB
            tc.tile_pool(name="tmp", bufs=8) as tmpp,     # rope temporaries
            tc.tile_pool(name="rt", bufs=4) as rtp,       # rope outputs
            tc.tile_pool(name="pp", bufs=3) as ppool,     # P stripes
            tc.tile_pool(name="rc", bufs=2) as rcp,       # reciprocal out
            tc.tile_pool(name="outp", bufs=4) as outp,    # c_proj staging
            tc.tile_pool(name="ps_a", bufs=3, space="PSUM") as pool_a,
            tc.tile_pool(name="ps_sg", bufs=3, space="PSUM") as pool_sg,
            tc.tile_pool(name="ps_dn", bufs=2, space="PSUM") as pool_dn,
        ):
            # ---- constants ----
            # wqk on sync (k-major: stage A consumes k-tiles in order);
            # everything else issued from other sequencers -- each dma_start
            # costs ~0.6us of serial issue time on its sequencer.
            wqk_s = constp.tile([128, NKT, 4 * 128], BF16, tag="wqk")
            for k in range(NKT):
                nc.sync.dma_start(wqk_s[:, k, :], wqk[k])
            x0_tiles = []
            for k2 in range(NKT // 2):
                xt = xp.tile([128, 2, HBS], BF16, tag="x", name="xt0")
                nc.scalar.dma_start(
                    xt, xP[0, 2 * k2 : 2 * k2 + 2].rearrange("k p t -> p k t")
                )
                x0_tiles.append(xt)
            wv_s = constp.tile([128, NKT, HPC * D], BF16, tag="wv")
            for k4 in range(4):
                nc.scalar.dma_start(
                    wv_s[:, k4 * 4 : (k4 + 1) * 4, :],
                    wv[k4 * 4 : (k4 + 1) * 4].rearrange("k p t -> p k t"),
                )
            cs_s = constp.tile([128, T], BF16, tag="cs")
            sn_s = constp.tile([128, T], BF16, tag="sn")
            nc.scalar.dma_start(
                cs_s.rearrange("p (q t) -> p q t", q=4),
                cs[:, :, :].rearrange("q p t -> p q t"),
            )
            nc.scalar.dma_start(
                sn_s.rearrange("p (q t) -> p q t", q=4),
                sn[:, :, :].rearrange("q p t -> p q t"),
            )
            ones_s = constp.tile([128, 128], BF16, tag="ones")
            nc.scalar.dma_start(ones_s, ones_sq[:, :])
            wp_s = constp.tile([128, HPC, C], BF16, tag="wp")
            for hh in range(HPC):
                nc.scalar.dma_start(
                    wp_s[:, hh, :].rearrange("p (n t) -> p n t", n=4),
                    wp[hh].rearrange("n p t -> p n t"),
                )

            # per-batch activation tiles (bufs=2 cycles across batches)
            QKH = [None] * B     # [128, 2(q|k), HPC, T]
            VH = [None] * B
            YT = [None] * B

            def stage_a(u):
                """qkv + rope for query block u (two 256-token half-blocks)."""
                b, tbl = divmod(u, NTB_B)
                if tbl == 0:
                    QKH[b] = qkp.tile([128, 2, HPC, T], BF16, tag="QKH", name="QKH")
                    VH[b] = vpool.tile([128, NKT, HPC * D], BF16, tag="VH", name="VH")
                    YT[b] = ypool.tile([128, HPC, T], BF16, tag="yT", name="yT")
                for half in range(2):
                    hb = tbl * 2 + half             # half-block in batch, 0..7
                    hbg = u * 2 + half              # global half-block, 0..15
                    if hbg == 0:
                        xt2s = x0_tiles
                    else:
                        xt2s = []
                        for k2 in range(NKT // 2):
                            xt = xp.tile([128, 2, HBS], BF16, tag="x")
                            nc.sync.dma_start(
                                xt,
                                xP[hbg, 2 * k2 : 2 * k2 + 2].rearrange("k p t -> p k t"),
                            )
                            xt2s.append(xt)

                    def xk(k):
                        return xt2s[k // 2][:, k % 2, :]
                    qkT = pool_a.tile([128, 512], F32, tag="a")  # [Qtop|Ktop]
                    qkB = pool_a.tile([128, 512], F32, tag="a")  # [Qbot|Kbot]
                    vps = pool_a.tile([128, 512], F32, tag="a")  # 2 x [tok128, 256]
                    for k in range(NKT):
                        # qkT/qkB are single PSUM banks holding two regions
                        # (Q cols 0:HBS, K cols HBS:512): exactly one start
                        # (first write, zeroes the bank's has_written bits)
                        # and one stop (last write) per bank.
                        st, sp = (k == 0), (k == NKT - 1)
                        nc.tensor.matmul(qkT[:, 0:HBS], wqk_s[:, k, 0:128], xk(k), start=st, stop=False)
                        nc.tensor.matmul(qkB[:, 0:HBS], wqk_s[:, k, 128:256], xk(k), start=st, stop=False)
                        nc.tensor.matmul(qkT[:, HBS:512], wqk_s[:, k, 256:384], xk(k), start=False, stop=sp)
                        nc.tensor.matmul(qkB[:, HBS:512], wqk_s[:, k, 384:512], xk(k), start=False, stop=sp)
                    for k in range(NKT):
                        for s in range(2):
                            # vps is one PSUM bank: one start (clears the
                            # bank), per-element has_written makes the first
                            # write of each region a plain store.
                            nc.tensor.matmul(
                                vps[:, s * 256 : (s + 1) * 256],
                                xk(k)[:, s * 128 : (s + 1) * 128],
                                wv_s[:, k, :],
                                start=(k == 0 and s == 0),
                                stop=(k == NKT - 1 and s == 1),
                            )
                    # evict qk psum fast (ACT), rope on DVE in bf16
                    qkTs = qkep.tile([128, 512], BF16, tag="qke")
                    nc.scalar.activation(qkTs, qkT, mybir.ActivationFunctionType.Copy)
                    qkBs = qkep.tile([128, 512], BF16, tag="qke")
                    nc.scalar.activation(qkBs, qkB, mybir.ActivationFunctionType.Copy)
                    # v eviction psum -> sbuf bf16
                    for s in range(2):
                        nc.scalar.activation(
                            VH[b][:, hb * 2 + s, :],
                            vps[:, s * 256 : (s + 1) * 256],
                            mybir.ActivationFunctionType.Copy,
                        )
                    c_b = cs_s[:, hb * HBS : (hb + 1) * HBS][:, None, :].broadcast_to([128, 2, HBS])
                    s_b = sn_s[:, hb * HBS : (hb + 1) * HBS][:, None, :].broadcast_to([128, 2, HBS])
                    qkT2 = qkTs.rearrange("p (a n) -> p a n", a=2)
                    qkB2 = qkBs.rearrange("p (a n) -> p a n", a=2)
                    t1 = tmpp.tile([128, 2, HBS], BF16, tag="t")
                    nc.vector.tensor_mul(t1, qkT2, c_b)
                    t2 = tmpp.tile([128, 2, HBS], BF16, tag="t")
                    nc.vector.tensor_mul(t2, qkB2, s_b)
                    t3 = tmpp.tile([128, 2, HBS], BF16, tag="t")
                    nc.vector.tensor_mul(t3, qkT2, s_b)
                    t4 = tmpp.tile([128, 2, HBS], BF16, tag="t")
                    nc.vector.tensor_mul(t4, qkB2, c_b)
                    rtop = rtp.tile([128, 2, HBS], BF16, tag="rt")
                    nc.vector.tensor_sub(rtop, t1, t2)
                    rbot = rtp.tile([128, 2, HBS], BF16, tag="rt")
                    nc.vector.tensor_add(rbot, t3, t4)
                    # regather into per-head layout: head-0 tops and head-1
                    # bots stay on their partitions (DVE copy); the other two
                    # quarters cross partitions (DMA, issued from gpsimd).
                    tcols = slice(hb * HBS, (hb + 1) * HBS)
                    nc.vector.tensor_copy(QKH[b][0:64, :, 0, tcols], rtop[0:64, :, :])
                    nc.vector.tensor_copy(QKH[b][64:128, :, 1, tcols], rbot[64:128, :, :])
                    nc.scalar.dma_start(QKH[b][64:128, :, 0, tcols], rbot[0:64, :, :])
                    nc.scalar.dma_start(QKH[b][0:64, :, 1, tcols], rtop[64:128, :, :])

            def stage_b(u):
                """causal attention for query block u (both heads)."""
                b, j = divmod(u, NTB_B)
                n_k = 4 * (j + 1)
                for h in range(HPC):
                    stripes = []
                    for _si in range((n_k + 7) // 8):
                        p_stripe = ppool.tile([128, 4096], BF16, tag="P")
                        stripes.append(p_stripe)

                    def poff(m):
                        return 128 * max(0, m - 4 * j)

                    def pchunk(m):
                        o = poff(m)
                        return stripes[m // 8][:, (m % 8) * 512 + o : (m % 8) * 512 + 512]

                    den = pool_dn.tile([128, 512], F32, tag="d")
                    pv = pool_a.tile([128, 512], F32, tag="a")

                    def denpv_pair(g):
                        for w in (0, 1):
                            m = 2 * g + w
                            o = poff(m)
                            nc.tensor.matmul(
                                den[:, o:512], ones_s, pchunk(m),
                                start=(m == 0), stop=(m == n_k - 1),
                            )
                            nc.tensor.matmul(
                                pv[:, o:512], VH[b][:, m, h * D : (h + 1) * D], pchunk(m),
                                start=(m == 0), stop=(m == n_k - 1),
                            )

                    for g in range(n_k // 2):
                        for v in (0, 1):
                            m = 2 * g + v
                            o = poff(m)
                            sg = pool_sg.tile([128, 512], F32, tag="sg")
                            nc.tensor.matmul(
                                sg[:, o:512],
                                QKH[b][:, 1, h, m * 128 : (m + 1) * 128],
                                QKH[b][:, 0, h, j * TBS + o : (j + 1) * TBS],
                                start=True,
                                stop=True,
                            )
                            nc.scalar.activation(
                                stripes[m // 8][:, (m % 8) * 512 + o : (m % 8) * 512 + 512],
                                sg[:, o:512],
                                mybir.ActivationFunctionType.Exp, scale=INV_SQRT_D,
                            )
                        # causal mask on diagonal chunks: only the 128-col
                        # sub-block at the chunk's own diagonal needs masking
                        for v in (0, 1):
                            m = 2 * g + v
                            r = m - 4 * j
                            if r >= 0:
                                ck = stripes[m // 8][
                                    :, (m % 8) * 512 + 128 * r : (m % 8) * 512 + 128 * r + 128
                                ]
                                nc.gpsimd.affine_select(
                                    out=ck,
                                    in_=ck,
                                    compare_op=mybir.AluOpType.is_ge,
                                    fill=0.0,
                                    base=0,
                                    pattern=[[1, 128]],
                                    channel_multiplier=-1,
                                )
                        if g >= 2:
                            denpv_pair(g - 2)
                    if n_k // 2 >= 2:
                        denpv_pair(n_k // 2 - 2)
                    denpv_pair(n_k // 2 - 1)
                    rc = rcp.tile([128, 512], F32, tag="rc")
                    nc.vector.reciprocal_approx_fast(out=rc, in_=den)
                    qsl = slice(j * TBS, (j + 1) * TBS)
                    nc.vector.tensor_mul(YT[b][:, h, qsl], pv, rc)

            def stage_c(u):
                """partial c_proj for query block u."""
                b, j = divmod(u, NTB_B)
                for i4 in range(4):
                    i = j * 4 + i4
                    row0 = b * T + i * 128
                    for n2 in range(C // 512):
                        ps = pool_a.tile([128, 512], F32, tag="a")
                        for hh in range(HPC):
                            nc.tensor.matmul(
                                ps,
                                YT[b][:, hh, i * 128 : (i + 1) * 128],
                                wp_s[:, hh, n2 * 512 : (n2 + 1) * 512],
                                start=(hh == 0),
                                stop=(hh == HPC - 1),
                            )
                        ot = outp.tile([128, 512], F32, tag="o")
                        if n2 % 2 == 0:
                            nc.scalar.activation(
                                ot, ps, mybir.ActivationFunctionType.Copy
                            )
                            nc.scalar.dma_start(
                                out_d[row0 : row0 + 128, n2 * 512 : (n2 + 1) * 512], ot
                            )
                        else:
                            nc.vector.tensor_copy(ot, ps)
                            nc.sync.dma_start(
                                out_d[row0 : row0 + 128, n2 * 512 : (n2 + 1) * 512], ot
                            )

            for step in range(NU + 2):
                if step < NU:
                    stage_a(step)
                if 1 <= step <= NU:
                    stage_b(step - 1)
                if step >= 2:
                    stage_c(step - 2)

    nc.compile()
    return nc


def _host_prep(x, w_atten, w_proj):
    """Build the shared + per-core input arrays."""
    x = np.asarray(x, dtype=np.float32)
    w_atten = np.asarray(w_atten, dtype=np.float32)
    w_proj = np.asarray(w_proj, dtype=np.float32)

    # xP[hbg, k, p, t'] = x[token hbg*HBS+t', channel k*128+p]: every
    # [128, HBS] tile DMA reads one contiguous 64KB DRAM chunk.
    xP = np.ascontiguousarray(
        x.reshape(BT // HBS, HBS, NKT, 128).transpose(0, 2, 3, 1).astype(NP_BF16)
    )

    wq = w_atten[0:C]
    wk = w_atten[C : 2 * C]
    wv_full = w_atten[2 * C : 3 * C]

    # rope tables: theta_i = base^(-2i/D)
    theta = 1.0 / (10000.0 ** (np.arange(0, D, 2, dtype=np.float64) / D))  # [64]
    tpos = np.arange(T, dtype=np.float64)
    ang = np.outer(theta, tpos)  # [64, T]
    cs_half = np.cos(ang).astype(np.float32)
    sn_half = np.sin(ang).astype(np.float32)
    cs = np.concatenate([cs_half, cs_half], axis=0).astype(NP_BF16)  # [128, T]
    sn = np.concatenate([sn_half, sn_half], axis=0).astype(NP_BF16)
    cs = np.ascontiguousarray(cs.reshape(128, 4, 512).transpose(1, 0, 2))
    sn = np.ascontiguousarray(sn.reshape(128, 4, 512).transpose(1, 0, 2))

    ones_sq = np.ones((128, 128), dtype=np.float32).astype(NP_BF16)

    top_idx = np.arange(0, D, 2)   # 64
    bot_idx = np.arange(1, D, 2)

    in_maps = []
    for c in range(NC_CORES):
        heads = [HPC * c + h for h in range(HPC)]
        # fb0 (tops of q), fb1 (bots of q), fb2/fb3 same for k
        fb = []
        for wmat in (wq, wk):
            for idx in (top_idx, bot_idx):
                rows = np.concatenate([wmat[hh * D + idx] for hh in heads], axis=0)
                fb.append(rows)  # [128, C]
        w_qk_c = np.concatenate(fb, axis=0)  # [512, C]
        wqk_dev = np.ascontiguousarray(
            w_qk_c.T.reshape(NKT, 128, 4 * 128).astype(NP_BF16)
        )
        w_v_c = np.concatenate([wv_full[hh * D : (hh + 1) * D] for hh in heads], axis=0)
        wv_dev = np.ascontiguousarray(
            w_v_c.T.reshape(NKT, 128, HPC * D).astype(NP_BF16)
        )
        cols = np.concatenate([np.arange(hh * D, (hh + 1) * D) for hh in heads])
        w_p_c = np.ascontiguousarray(w_proj[:, cols].T)  # [256, C]
        wp_dev = np.ascontiguousarray(
            w_p_c.reshape(HPC, 128, C // 512, 512).transpose(0, 2, 1, 3).astype(NP_BF16)
        )
        in_maps.append(
            {
                "xP": xP,
                "wqk": wqk_dev,
                "wv": wv_dev,
                "wp": wp_dev,
                "cs": cs,
                "sn": sn,
                "ones_sq": ones_sq,
            }
        )
    return in_maps


def _execute(in_maps, trace=False, trace_kwargs=None):
    if "nc" not in _CACHE:
        _CACHE["nc"] = _build_program()
    nc = _CACHE["nc"]
    kwargs = {}
    if trace:
        _install_ntff_hook()
        kwargs["trace"] = True
        if trace_kwargs:
            kwargs.update(trace_kwargs)
    return run_bass_kernel_spmd(nc, in_maps, core_ids=list(range(NC_CORES)), **kwargs)


def _install_ntff_hook():
    """Restore the axon NTFF profile hook (the container's antenv lacks it)."""
    import types

    if "antenv.axon_hooks" in sys.modules:
        return
    mod = types.ModuleType("antenv.axon_hooks")
    mod._hook = None

    def set_axon_ntff_profile_hook(h):
        mod._hook = h

    def get_axon_ntff_profile_hook():
        if mod._hook is None:
            try:
                from trn_agent_boot.trn_boot import _ntff_profile_via_ctypes

                mod._hook = _ntff_profile_via_ctypes("/opt/axon/libaxon_pjrt.so")
            except Exception:
                mod._hook = None
        return mod._hook

    mod.set_axon_ntff_profile_hook = set_axon_ntff_profile_hook
    mod.get_axon_ntff_profile_hook = get_axon_ntff_profile_hook
    sys.modules["antenv.axon_hooks"] = mod


def kernel(x, w_atten, w_proj):
    in_maps = _host_prep(x, w_atten, w_proj)
    res = _execute(in_maps)
    total = res.results[0]["out"].astype(np.float32)
    for c in range(1, NC_CORES):
        total = total + res.results[c]["out"]
    return total.reshape(B, T, C)
